# revision 15
# baseline (speedup 1.0000x reference)
"""Self-contained Bass/Tile GRU kernel for Trainium2 (8 NeuronCores).

Problem: CustomGRU forward, SEQ=2048, BATCH=32, INPUT=HIDDEN=256.
Sharding: data-parallel over batch (4 rows per core), weights replicated,
sequential scan local per core.

Layout strategy (per core, B=4 local batch):
  Everything lives transposed: hidden/gate units on SBUF partitions, batch on
  the (tiny) free dim.  That makes every elementwise op a [128, small] op and
  the recurrent matmul a weights-stationary matmul:
      gatesT[g, b] = sum_k W_hh[g, k] * hT[k, b]   (lhsT = W_hh^T tiles)
  x-projections are precomputed with one big GEMM into DRAM scratch
  (xpT[m, p, s, b]), then streamed into SBUF chunk-by-chunk during the scan.
  The n-gate identity used: with m1 = h @ W_hh^T,
      n = tanh(xp_n + m1_n + r * m1_n)
  so no second matmul with W_hn is needed.
"""

import numpy as np

import concourse.bacc as bacc
import concourse.bass as bass
import concourse.mybir as mybir
import concourse.tile as tile
from concourse import bass_utils
from concourse.bass import ds
from concourse.masks import make_identity

SEQ, BATCH, INPUT, HIDDEN = 2048, 32, 256, 256
NCORES = 8
F32 = mybir.dt.float32
AF = mybir.ActivationFunctionType


def build_gru(S=SEQ, Bl=BATCH // NCORES, CH=128, unroll_chunks=2, repeats=1,
              state_fp16=True):
    """Build the single-core Bass program (same program runs SPMD on 8 cores).

    S: sequence length, Bl: local batch, CH: steps per chunk.
    repeats: emit the whole compute `repeats` times (for differential timing).
    state_fp16: keep W_hh^T and the hidden state in fp16 (halves LDWEIGHTS
    via fast-weight-load; ~5e-4 relative noise, washed out by the gates).
    """
    H, I = HIDDEN, INPUT
    G = 3 * H
    KT = H // 128     # 2 k-tiles (contraction over hidden)
    IT = I // 128     # 2 k-tiles (contraction over input)
    MT = G // 128     # 6 m-tiles (gate rows)
    NCH = S // CH     # chunks
    RB = CH * Bl      # x rows per chunk (= 512 for full size)
    assert S % CH == 0 and NCH % unroll_chunks == 0
    assert RB <= 512

    nc = bacc.Bacc("TRN2", target_bir_lowering=False, debug=False)

    x_d = nc.dram_tensor("input", [S, Bl, I], F32, kind="ExternalInput")
    wih_d = nc.dram_tensor("W_ih", [G, I], F32, kind="ExternalInput")
    whh_d = nc.dram_tensor("W_hh", [G, H], F32, kind="ExternalInput")
    bih_d = nc.dram_tensor("b_ih", [G], F32, kind="ExternalInput")
    bhh_d = nc.dram_tensor("b_hh", [G], F32, kind="ExternalInput")
    # outT[k, p, s, b] = h_s[b, k*128+p]
    outT_d = nc.dram_tensor("outT", [KT, 128, S, Bl], F32, kind="ExternalOutput")
    # xpT[m, p, s, b] = (x_s,b @ W_ih^T + b_ih + b_hh)[m*128+p]
    xpT_d = nc.dram_tensor("xpT", [MT, 128, S, Bl], F32)

    x_flat = x_d.ap().flatten_outer_dims()        # [S*Bl, I]
    xpT = xpT_d.ap()
    outT = outT_d.ap()

    with tile.TileContext(nc) as tc:
        with tc.tile_pool(name="const", bufs=1) as cpool:
            # ---- Phase A: identity, transposed weights, bias ----
            ident = cpool.tile([128, 128], F32, tag="ident")
            make_identity(nc, ident)

            st_dt = mybir.dt.float16 if state_fp16 else F32
            wihT = cpool.tile([128, IT, G], F32, tag="wihT")   # [p=i, k, g]
            whhT = cpool.tile([128, KT, G], st_dt, tag="whhT")  # [p=h, k, g]
            bias = cpool.tile([128, MT], F32, tag="bias")

            with (
                tc.tile_pool(name="wprep", bufs=3) as wpool,
                tc.tile_pool(name="wprep_ps", bufs=3, space="PSUM") as wpool_ps,
            ):
                for (w_src, w_dst, kt) in ((wih_d, wihT, IT), (whh_d, whhT, KT)):
                    for gb in range(MT):
                        w_nat = wpool.tile([128, 256], F32, tag="w_nat")
                        nc.sync.dma_start(w_nat, w_src.ap()[gb * 128:(gb + 1) * 128, :])
                        for k in range(kt):
                            ps = wpool_ps.tile([128, 128], F32, tag="w_ps")
                            nc.tensor.transpose(ps, w_nat[:, k * 128:(k + 1) * 128], ident)
                            nc.vector.tensor_copy(
                                w_dst[:, k, gb * 128:(gb + 1) * 128], ps)
                bsum_a = wpool.tile([128, MT], F32, tag="bsum_a")
                bsum_b = wpool.tile([128, MT], F32, tag="bsum_b")
                nc.sync.dma_start(bsum_a, bih_d.ap().rearrange("(m p) -> p m", p=128))
                nc.sync.dma_start(bsum_b, bhh_d.ap().rearrange("(m p) -> p m", p=128))
                nc.vector.tensor_add(bias, bsum_a, bsum_b)

            for _rep in range(repeats):
                _emit_compute(nc, tc, ident, wihT, whhT, bias, x_flat, xpT, outT,
                              S, Bl, CH, unroll_chunks, st_dt)

    nc.compile()
    return nc


def _emit_compute(nc, tc, ident, wihT, whhT, bias, x_flat, xpT, outT,
                  S, Bl, CH, unroll_chunks, st_dt):
    H, I = HIDDEN, INPUT
    G = 3 * H
    KT = H // 128
    IT = I // 128
    MT = G // 128
    NCH = S // CH
    RB = CH * Bl
    if True:
        if True:
            # ---- Phase B: xpT = W_ih @ x^T + bias, chunk by chunk ----
            with (
                tc.tile_pool(name="pb_sb", bufs=3) as pbs,
                tc.tile_pool(name="pb_xt", bufs=2) as pbxt,
                tc.tile_pool(name="pb_ps", bufs=2, space="PSUM") as pbps,
                tc.tile_pool(name="pb_ps2", bufs=2, space="PSUM") as pbps2,
            ):
                n_sub = (RB + 127) // 128
                for pc in range(NCH):
                    xT = pbxt.tile([128, IT, RB], F32, tag="xT")
                    for t in range(n_sub):
                        r0 = pc * RB + t * 128
                        rt = min(128, S * Bl - r0, RB - t * 128)
                        x_nat = pbs.tile([128, I], F32, tag="x_nat")
                        nc.sync.dma_start(x_nat[:rt, :], x_flat[r0:r0 + rt, :])
                        for k in range(IT):
                            ps = pbps.tile([128, 128], F32, tag="tr_ps")
                            nc.tensor.transpose(
                                ps[:, :rt], x_nat[:rt, k * 128:(k + 1) * 128],
                                ident[:rt, :rt])
                            nc.vector.tensor_copy(
                                xT[:, k, t * 128:t * 128 + rt], ps[:, :rt])
                    for m in range(MT):
                        psx = pbps2.tile([128, RB], F32, tag="xp_ps")
                        for k in range(IT):
                            nc.tensor.matmul(
                                psx, wihT[:, k, m * 128:(m + 1) * 128], xT[:, k, :],
                                start=(k == 0), stop=(k == IT - 1))
                        xp_sb = pbs.tile([128, RB], F32, tag="xp_sb")
                        nc.scalar.activation(
                            xp_sb, psx, AF.Identity, bias=bias[:, m:m + 1], scale=1.0)
                        nc.sync.dma_start(
                            xpT[m, :, ds(pc * CH, CH), :],
                            xp_sb.rearrange("p (s b) -> p s b", b=Bl))

            # ---- Phase C: the scan ----
            with (
                tc.tile_pool(name="scan", bufs=1) as spool,
                tc.tile_pool(name="step", bufs=4) as stp,
                tc.tile_pool(name="step_ps", bufs=3, space="PSUM") as spsum,
            ):
                xp_buf = [spool.tile([128, MT, CH, Bl], F32, tag=f"xp{i}",
                                     name=f"xp{i}")
                          for i in range(2)]
                hist = [spool.tile([128, KT, CH, Bl], st_dt, tag=f"hist{i}",
                                   name=f"hist{i}")
                        for i in range(2)]

                def load_xp(buf, c_expr):
                    for m in range(MT):
                        nc.sync.dma_start(
                            buf[:, m, :, :], xpT[m, :, ds(c_expr * CH, CH), :])

                def store_hist(buf, c_expr):
                    # gpsimd DMA casts fp16 hist -> fp32 outT when needed
                    dma = nc.sync if st_dt == F32 else nc.gpsimd
                    for k in range(KT):
                        dma.dma_start(
                            outT[k, :, ds(c_expr * CH, CH), :], buf[:, k, :, :])

                # h = 0 before the first step; chunk 0 reads hist[1]'s last slice.
                nc.vector.memset(hist[1][:, :, CH - 1, :], 0.0)
                load_xp(xp_buf[0], 0)

                def gru_step(xp, hst, h_prev):
                    """One GRU step. xp: [128,MT,Bl] this step's x-proj slice,
                    hst: [128,KT,Bl] destination for h', h_prev: [128,KT,Bl]."""
                    psum = spsum.tile([128, MT, Bl], F32, tag="gates")
                    # `start=True` clears has_written for the WHOLE bank, so
                    # exactly the first matmul into this tile uses it; all
                    # later matmuls rely on per-element has_written bits
                    # (first write to a region overwrites, later ones
                    # accumulate).
                    # r/z gate tiles: init with xp via identity matmul.
                    for m in range(4):
                        nc.tensor.matmul(psum[:, m, :], ident, xp[:, m, :],
                                         start=(m == 0), stop=False,
                                         skip_group_check=True)
                    # n-gate tiles (kept raw: no xp folded in)
                    for m in (4, 5):
                        for k in range(KT):
                            nc.tensor.matmul(
                                psum[:, m, :], whhT[:, k, m * 128:(m + 1) * 128],
                                h_prev[:, k, :],
                                start=False, stop=False, skip_group_check=True)
                    for m in range(4):
                        for k in range(KT):
                            nc.tensor.matmul(
                                psum[:, m, :], whhT[:, k, m * 128:(m + 1) * 128],
                                h_prev[:, k, :],
                                start=False, stop=(m == 3 and k == KT - 1),
                                skip_group_check=True)
                    # Only Tanh is used on ScalarE (one activation table set;
                    # switching sets costs ~2.7us).  sigmoid(x) folded as
                    # 0.5*tanh(x/2)+0.5 into the downstream fused DVE ops:
                    #   r*m1n          = (t_r+1) * (0.5*m1n)
                    #   arg            = r*m1n + m1n + xp_n
                    #                  = t_r*(0.5*m1n) + (1.5*m1n + xp_n)
                    #   h' = (1-z)n+zh = (0.5-0.5*t_z)*n + 0.5*(t_z+1)*h
                    m1h = stp.tile([128, 2, Bl], F32, tag="m1h")
                    nc.scalar.mul(m1h, psum[:, 4:6, :], 0.5)
                    a2 = stp.tile([128, 2, Bl], F32, tag="a2")
                    nc.vector.scalar_tensor_tensor(
                        a2, psum[:, 4:6, :], 1.5, xp[:, 4:6, :],
                        mybir.AluOpType.mult, mybir.AluOpType.add)
                    trz = stp.tile([128, 4, Bl], F32, tag="trz")
                    nc.scalar.activation(trz, psum[:, 0:4, :], AF.Tanh, scale=0.5)
                    u = stp.tile([128, 2, Bl], F32, tag="u")
                    nc.vector.tensor_mul(u, trz[:, 0:2, :], m1h)
                    arg = stp.tile([128, 2, Bl], F32, tag="arg")
                    nc.vector.tensor_add(arg, u, a2)
                    n_sb = stp.tile([128, 2, Bl], F32, tag="n_sb")
                    nc.scalar.activation(n_sb, arg, AF.Tanh)
                    # off the critical path (independent of n):
                    #   p = (t_z+1)*h,  q = 0.5 - 0.5*t_z
                    p = stp.tile([128, 2, Bl], F32, tag="p")
                    nc.vector.scalar_tensor_tensor(
                        p, trz[:, 2:4, :], 1.0, h_prev,
                        mybir.AluOpType.add, mybir.AluOpType.mult)
                    q = stp.tile([128, 2, Bl], F32, tag="q")
                    nc.vector.tensor_scalar(
                        q, trz[:, 2:4, :], -0.5, 0.5,
                        mybir.AluOpType.mult, mybir.AluOpType.add)
                    # after n: h' = q*n + 0.5*p   (2 serial DVE ops)
                    qn = stp.tile([128, 2, Bl], F32, tag="qn")
                    nc.vector.tensor_mul(qn, q, n_sb)
                    nc.vector.scalar_tensor_tensor(
                        hst, p, 0.5, qn,
                        mybir.AluOpType.mult, mybir.AluOpType.add)

                def do_chunk(par, prev_par, prefetch_c=None, store_c=None):
                    """par: buffer parity of this chunk."""
                    xp = xp_buf[par]
                    hst = hist[par]
                    prev = hist[prev_par]
                    for s_ in range(CH):
                        if s_ == CH // 2 and prefetch_c is not None:
                            load_xp(xp_buf[prev_par], prefetch_c)
                        h_prev = (prev[:, :, CH - 1, :] if s_ == 0
                                  else hst[:, :, s_ - 1, :])
                        gru_step(xp[:, :, s_, :], hst[:, :, s_, :], h_prev)
                    if store_c is not None:
                        store_hist(hst, store_c)

                U = unroll_chunks
                with tc.For_i(0, NCH // U) as it:
                    # chunks c0 = U*it + 0, ..., U*it + (U-1)
                    for j in range(U):
                        cj = it * U + j
                        do_chunk(
                            par=j % 2,
                            prev_par=(j + 1) % 2,
                            prefetch_c=None,
                            store_c=cj,
                        )
                        # prefetch the next chunk into the other buffer; the
                        # scheduler overlaps it with this chunk's compute
                        # (no data dependency).  On the very last chunk this
                        # wraps to chunk 0 -- an in-bounds, harmless refetch.
                        nxt = (it * U + j + 1) % NCH
                        load_xp(xp_buf[(j + 1) % 2], nxt)


_CACHE = {}


def _get_nc():
    if "nc" not in _CACHE:
        _CACHE["nc"] = build_gru()
    return _CACHE["nc"]


def kernel(input, W_ih, W_hh, b_ih, b_hh):
    input = np.ascontiguousarray(np.asarray(input, dtype=np.float32))
    W_ih = np.ascontiguousarray(np.asarray(W_ih, dtype=np.float32))
    W_hh = np.ascontiguousarray(np.asarray(W_hh, dtype=np.float32))
    b_ih = np.ascontiguousarray(np.asarray(b_ih, dtype=np.float32))
    b_hh = np.ascontiguousarray(np.asarray(b_hh, dtype=np.float32))

    nc = _get_nc()
    Bl = BATCH // NCORES
    in_maps = [
        {
            "input": np.ascontiguousarray(input[:, j * Bl:(j + 1) * Bl, :]),
            "W_ih": W_ih,
            "W_hh": W_hh,
            "b_ih": b_ih,
            "b_hh": b_hh,
        }
        for j in range(NCORES)
    ]
    res = bass_utils.run_bass_kernel_spmd(nc, in_maps, core_ids=list(range(NCORES)))
    outs = []
    for j in range(NCORES):
        outT = res.results[j]["outT"]            # [2, 128, S, Bl]
        outs.append(outT.reshape(HIDDEN, SEQ, Bl).transpose(1, 2, 0))
    output = np.concatenate(outs, axis=1)        # [S, B, H]
    h_n = output[-1][None, :, :]
    return output, h_n
